# revision 8
# baseline (speedup 1.0000x reference)
"""GRU encoder kernel for trn2: 8-way batch-data-parallel Bass/Tile kernel.

Problem: B=64, T=512, F=64, H=256 (rnn), D=64 (dense).
  xproj = x @ kernel + bias                      [B, T, 3H]
  per t: rec = h @ recurrent_kernel              [B, 3H]
         z = sigmoid(xz + rz); r = sigmoid(xr + rr)
         hh = tanh(xh + r * rh)
         h  = z*h + (1-z)*hh
  outputs[:, t] = h; state = tanh(h_T @ dense_w + dense_b)

Sharding: batch 64 -> 8 cores x 8 rows, weights replicated, no collectives.
"""

import os
import sys
from contextlib import ExitStack

import numpy as np

import concourse.bass as bass
import concourse.mybir as mybir
import concourse.tile as tile
from concourse.bass_utils import run_bass_kernel_spmd
from concourse.masks import make_identity

# Problem constants (hardcoded per contract).
B_FULL, T, F, H, D = 64, 512, 64, 64 * 4, 64  # H=256
G3 = 3 * H  # 768
NCORES = 8
B = B_FULL // NCORES  # 8 per core

FP = mybir.dt.float32
CHUNK = 8  # timesteps per xp-prefetch / out-ring chunk
N_CHUNKS = T // CHUNK

_CACHE = {}


def _split_waits(nc, template, cap=1):
    """walrus codegen rejects instructions with more sync waits than the
    engine's ISA struct supports (Matmult/LDW allows only one). Move excess
    waits onto cloned same-engine NOPs inserted right before the instruction
    (same per-engine queue order, identical semantics)."""
    import copy

    nid = 0
    for f in nc.m.functions:
        for bb in f.blocks:
            new = []
            for inst in bb.instructions:
                si = inst.sync_info
                waits = list(si.on_wait) if (si and si.on_wait) else []
                if len(waits) > cap and type(inst).__name__ != "InstNoOp":
                    for w in waits[:-cap]:
                        nid += 1
                        nop = copy.deepcopy(template)
                        nop.name = f"I-wsplit-{nid}"
                        nop.engine = inst.engine
                        nop.sync_info = mybir.SyncInfo(on_wait=[w], on_update=[])
                        new.append(nop)
                    inst.sync_info = mybir.SyncInfo(
                        on_wait=waits[-cap:],
                        on_update=list(si.on_update) if si.on_update else [],
                    )
                new.append(inst)
            bb.instructions = new


def _build_bass():
    nc = bass.Bass()

    x_d = nc.dram_tensor("x", [B, T, F], FP, kind="ExternalInput")
    k_d = nc.dram_tensor("kernel", [F, G3], FP, kind="ExternalInput")
    r_d = nc.dram_tensor("rkernel", [H, G3], FP, kind="ExternalInput")
    b_d = nc.dram_tensor("bias", [1, G3], FP, kind="ExternalInput")
    dw_d = nc.dram_tensor("dense_w", [H, D], FP, kind="ExternalInput")
    db_d = nc.dram_tensor("dense_b", [1, D], FP, kind="ExternalInput")

    out_d = nc.dram_tensor("out", [B, T, H], FP, kind="ExternalOutput")
    st_d = nc.dram_tensor("state", [B, D], FP, kind="ExternalOutput")

    with ExitStack() as ctx:
        tc = ctx.enter_context(tile.TileContext(nc))
        nop_template = nc.tensor.nop().ins
        consts = ctx.enter_context(tc.tile_pool(name="consts", bufs=1))
        statep = ctx.enter_context(tc.tile_pool(name="statep", bufs=1))
        dram = ctx.enter_context(tc.tile_pool(name="dram", bufs=1, space="DRAM"))

        # ---- constants in SBUF ----
        i128 = consts.tile([128, 128], FP)
        make_identity(nc, i128)
        i8 = consts.tile([8, 8], FP)
        make_identity(nc, i8)

        w_aug = consts.tile([F + 1, G3], FP)  # [65, 768]: kernel rows + bias row
        nc.sync.dma_start(w_aug[:F, :], k_d[:, :])
        nc.sync.dma_start(w_aug[F : F + 1, :], b_d[:, :])

        r0 = consts.tile([128, G3], FP)
        r1 = consts.tile([128, G3], FP)
        nc.sync.dma_start(r0[:, :], r_d[0:128, :])
        nc.sync.dma_start(r1[:, :], r_d[128:256, :])

        dw0 = consts.tile([128, D], FP)
        dw1 = consts.tile([128, D], FP)
        nc.sync.dma_start(dw0[:, :], dw_d[0:128, :])
        nc.sync.dma_start(dw1[:, :], dw_d[128:256, :])
        db_bc = consts.tile([B, D], FP)
        nc.sync.dma_start(
            db_bc[:, :],
            bass.AP(tensor=db_d, offset=0, ap=[[0, B], [1, D]]),
        )

        # persistent recurrent state, transposed: hT[:, k*8:(k+1)*8] = h[:, 128k:128(k+1)].T
        hT = statep.tile([128, 2 * B], FP)
        nc.vector.memset(hT[:, :], 0.0)

        # DRAM scratch for xproj, stored [T, B, G3] so per-step slices are [B, G3]
        xp_dram = dram.tile([T, B, G3], FP)

        # ================= Phase 1: xproj =================
        with (
            tc.tile_pool(name="p1_in", bufs=3) as p1_in,
            tc.tile_pool(name="p1_xt", bufs=3) as p1_xt,
            tc.tile_pool(name="p1_ps", bufs=2, space="PSUM") as p1_ps,
            tc.tile_pool(name="p1_pst", bufs=2, space="PSUM") as p1_pst,
            tc.tile_pool(name="p1_st", bufs=3) as p1_st,
        ):
            x_flat = x_d  # [B, T, F]
            for b in range(B):
                for mt in range(T // 128):
                    t0 = mt * 128
                    xtile = p1_in.tile([128, F], FP)
                    nc.sync.dma_start(xtile[:, :], x_flat[b, t0 : t0 + 128, :])
                    ps_t = p1_pst.tile([F, 128], FP)
                    nc.tensor.transpose(ps_t[:, :], xtile[:, :], i128[:, :])
                    xt_aug = p1_xt.tile([F + 1, 128], FP)
                    nc.vector.tensor_copy(xt_aug[:F, :], ps_t[:, :])
                    nc.vector.memset(xt_aug[F : F + 1, :], 1.0)
                    ps_xp = p1_ps.tile([128, G3], FP)
                    nc.tensor.matmul(
                        ps_xp[:, 0:512], xt_aug[:, :], w_aug[:, 0:512],
                        start=True, stop=True,
                    )
                    nc.tensor.matmul(
                        ps_xp[:, 512:G3], xt_aug[:, :], w_aug[:, 512:G3],
                        start=True, stop=True,
                    )
                    xp_sb = p1_st.tile([128, G3], FP)
                    nc.vector.tensor_copy(xp_sb[:, :], ps_xp[:, :])
                    nc.sync.dma_start(xp_dram[t0 : t0 + 128, b, :], xp_sb[:, :])

        # ================= Phase 2: recurrence =================
        with (
            tc.tile_pool(name="xp", bufs=2) as xp_pool,
            tc.tile_pool(name="ring", bufs=2) as ring_pool,
            tc.tile_pool(name="gates", bufs=2) as gates,
            tc.tile_pool(name="ps_rec", bufs=2, space="PSUM") as ps_rec_pool,
            tc.tile_pool(name="ps_sm", bufs=2, space="PSUM") as ps_sm_pool,
        ):
            prev_h = None  # AP of previous step's h (B-layout)
            for c in range(N_CHUNKS):
                t0 = c * CHUNK
                xp_tile = xp_pool.tile([B, CHUNK, G3], FP)
                # DRAM [t, b, g] chunk -> SBUF [b, t_local, g]
                src = bass.AP(
                    tensor=xp_dram.tensor,
                    offset=xp_dram.offset + t0 * B * G3,
                    ap=[[G3, B], [B * G3, CHUNK], [1, G3]],
                )
                nc.sync.dma_start(xp_tile[:, :, :], src)
                ring = ring_pool.tile([B, CHUNK, H], FP)

                for tl in range(CHUNK):
                    t = t0 + tl
                    first = t == 0
                    ps = ps_rec_pool.tile([B, G3], FP)
                    if not first:
                        nc.tensor.matmul(
                            ps[:, 0:512], hT[:, 0:B], r0[:, 0:512],
                            start=True, stop=False,
                        )
                        nc.tensor.matmul(
                            ps[:, 512:G3], hT[:, 0:B], r0[:, 512:G3],
                            start=True, stop=False,
                        )
                        nc.tensor.matmul(
                            ps[:, 0:512], hT[:, B : 2 * B], r1[:, 0:512],
                            start=False, stop=False,
                        )
                        nc.tensor.matmul(
                            ps[:, 512:G3], hT[:, B : 2 * B], r1[:, 512:G3],
                            start=False, stop=True,
                        )
                        # += xproj for z,r region
                        nc.tensor.matmul(
                            ps[:, 0:512], i8[:, :], xp_tile[:, tl, 0:512],
                            start=False, stop=True,
                        )
                    else:
                        nc.tensor.matmul(
                            ps[:, 0:512], i8[:, :], xp_tile[:, tl, 0:512],
                            start=True, stop=True,
                        )

                    zr = gates.tile([B, 512], FP)
                    nc.scalar.activation(
                        zr[:, :], ps[:, 0:512],
                        mybir.ActivationFunctionType.Sigmoid,
                    )
                    tt = gates.tile([B, H], FP)
                    if not first:
                        v = gates.tile([B, H], FP)
                        nc.vector.tensor_mul(v[:, :], zr[:, H:512], ps[:, 512:G3])
                        w = gates.tile([B, H], FP)
                        nc.vector.tensor_add(w[:, :], v[:, :], xp_tile[:, tl, 512:G3])
                        nc.scalar.activation(
                            tt[:, :], w[:, :], mybir.ActivationFunctionType.Tanh
                        )
                    else:
                        nc.scalar.activation(
                            tt[:, :], xp_tile[:, tl, 512:G3],
                            mybir.ActivationFunctionType.Tanh,
                        )

                    d = gates.tile([B, H], FP)
                    if first:
                        nc.vector.tensor_scalar_mul(d[:, :], tt[:, :], -1.0)
                    else:
                        nc.vector.tensor_sub(d[:, :], prev_h, tt[:, :])
                    e = gates.tile([B, H], FP)
                    nc.vector.tensor_mul(e[:, :], zr[:, 0:H], d[:, :])
                    h_out = ring[:, tl, :]
                    nc.vector.tensor_add(h_out, e[:, :], tt[:, :])
                    prev_h = h_out

                    # retranspose h' -> hT for next matmul
                    ps_t = ps_sm_pool.tile([128, 2 * B], FP)
                    nc.tensor.transpose(ps_t[:, 0:B], ring[:, tl, 0:128], i8[:, :])
                    nc.tensor.transpose(
                        ps_t[:, B : 2 * B], ring[:, tl, 128:256], i8[:, :]
                    )
                    nc.vector.tensor_copy(hT[:, :], ps_t[:, :])

                nc.sync.dma_start(out_d[:, t0 : t0 + CHUNK, :], ring[:, :, :])

            # ================= Phase 3: dense head =================
            ps_st = ps_sm_pool.tile([B, D], FP, tag="ps_state")
            nc.tensor.matmul(ps_st[:, :], hT[:, 0:B], dw0[:, :], start=True, stop=False)
            nc.tensor.matmul(
                ps_st[:, :], hT[:, B : 2 * B], dw1[:, :], start=False, stop=False
            )
            nc.tensor.matmul(ps_st[:, :], i8[:, :], db_bc[:, :], start=False, stop=True)
            st_sb = statep.tile([B, D], FP)
            nc.scalar.activation(
                st_sb[:, :], ps_st[:, :], mybir.ActivationFunctionType.Tanh
            )
            nc.sync.dma_start(st_d[:, :], st_sb[:, :])

    _split_waits(nc, nop_template)
    return nc


def _get_runner():
    """Build nc + a cached jitted shard_map executable (no output donation so
    the zero-output buffers can be reused across timing reps)."""
    if "runner" in _CACHE:
        return _CACHE["runner"]
    import jax
    from jax.experimental.shard_map import shard_map
    from jax.sharding import Mesh, PartitionSpec

    from concourse import bass2jax as b2j

    nc = _CACHE.setdefault("nc", _build_bass())
    b2j.install_neuronx_cc_hook()

    in_names, out_names, out_avals, zero_outs = [], [], [], []
    partition_name = nc.partition_id_tensor.name if nc.partition_id_tensor else None
    for alloc in nc.m.functions[0].allocations:
        if not isinstance(alloc, mybir.MemoryLocationSet):
            continue
        name = alloc.memorylocations[0].name
        if alloc.kind == "ExternalInput":
            if name != partition_name:
                in_names.append(name)
        elif alloc.kind == "ExternalOutput":
            shape = tuple(alloc.tensor_shape)
            dtype = mybir.dt.np(alloc.dtype)
            out_names.append(name)
            out_avals.append(jax.core.ShapedArray(shape, dtype))
            zero_outs.append(np.zeros(shape, dtype))
    n_params = len(in_names)
    all_in_names = in_names + out_names
    if partition_name is not None:
        all_in_names.append(partition_name)

    def _body(*args):
        operands = list(args)
        if partition_name is not None:
            operands.append(b2j.partition_id_tensor())
        outs = b2j._bass_exec_p.bind(
            *operands,
            out_avals=tuple(out_avals),
            in_names=tuple(all_in_names),
            out_names=tuple(out_names),
            lowering_input_output_aliases=(),
            sim_require_finite=True,
            sim_require_nnan=True,
            nc=nc,
        )
        return tuple(outs)

    devices = jax.devices()[:NCORES]
    mesh = Mesh(np.asarray(devices), ("core",))
    n_outs = len(out_names)
    sharded = jax.jit(
        shard_map(
            _body,
            mesh=mesh,
            in_specs=(PartitionSpec("core"),) * (n_params + n_outs),
            out_specs=(PartitionSpec("core"),) * n_outs,
            check_rep=False,
        ),
        keep_unused=True,
    )
    runner = (sharded, in_names, out_names, zero_outs)
    _CACHE["runner"] = runner
    return runner


def _prep_inputs(x, kernel, recurrent_kernel, bias, dense_w, dense_b):
    x = np.ascontiguousarray(np.asarray(x, dtype=np.float32))
    k = np.ascontiguousarray(np.asarray(kernel, dtype=np.float32))
    rk = np.ascontiguousarray(np.asarray(recurrent_kernel, dtype=np.float32))
    bi = np.ascontiguousarray(np.asarray(bias, dtype=np.float32).reshape(1, G3))
    dw = np.ascontiguousarray(np.asarray(dense_w, dtype=np.float32))
    db = np.ascontiguousarray(np.asarray(dense_b, dtype=np.float32).reshape(1, D))
    per_core = {
        "x": x.reshape(B_FULL, T, F),  # sharded over axis0 by shard_map
        "kernel": np.concatenate([k] * NCORES, axis=0),
        "rkernel": np.concatenate([rk] * NCORES, axis=0),
        "bias": np.concatenate([bi] * NCORES, axis=0),
        "dense_w": np.concatenate([dw] * NCORES, axis=0),
        "dense_b": np.concatenate([db] * NCORES, axis=0),
    }
    return per_core


def _run(concat_inputs):
    import jax

    sharded, in_names, out_names, zero_outs = _get_runner()
    zeros_cat = [
        np.zeros((NCORES * z.shape[0], *z.shape[1:]), z.dtype) for z in zero_outs
    ]
    args = [concat_inputs[n] for n in in_names] + zeros_cat
    outs = sharded(*args)
    outs = [np.asarray(o) for o in outs]
    return dict(zip(out_names, outs))


def kernel(x, kernel, recurrent_kernel, bias, dense_w, dense_b, trace=False):
    concat = _prep_inputs(x, kernel, recurrent_kernel, bias, dense_w, dense_b)
    res = _run(concat)
    return res["out"], res["state"]


def timed_run(inputs, reps=10):
    """Time device-resident repeated executions; returns (best_ns, list_ns)."""
    import time as _time

    import jax

    concat = _prep_inputs(**inputs)
    sharded, in_names, out_names, zero_outs = _get_runner()
    zeros_cat = [
        np.zeros((NCORES * z.shape[0], *z.shape[1:]), z.dtype) for z in zero_outs
    ]
    args = [concat[n] for n in in_names] + zeros_cat
    dev_args = jax.device_put(args)
    jax.block_until_ready(dev_args)
    # warmup (includes compile on first call)
    jax.block_until_ready(sharded(*dev_args))
    times = []
    for _ in range(reps):
        t0 = _time.perf_counter()
        jax.block_until_ready(sharded(*dev_args))
        times.append((_time.perf_counter() - t0) * 1e9)
    return min(times), times


# revision 16
# speedup vs baseline: 1.2197x; 1.2197x over previous
"""GRU encoder kernel for trn2: 8-way batch-data-parallel Bass/Tile kernel.

Problem: B=64, T=512, F=64, H=256 (rnn), D=64 (dense).
  xproj = x @ kernel + bias                      [B, T, 3H]
  per t: rec = h @ recurrent_kernel              [B, 3H]
         z = sigmoid(xz + rz); r = sigmoid(xr + rr)
         hh = tanh(xh + r * rh)
         h  = z*h + (1-z)*hh
  outputs[:, t] = h; state = tanh(h_T @ dense_w + dense_b)

Sharding: batch 64 -> 8 cores x 8 rows, weights replicated, no collectives.
"""

import os
import sys
from contextlib import ExitStack

import numpy as np

import concourse.bass as bass
import concourse.mybir as mybir
import concourse.tile as tile
from concourse.masks import make_identity

# Problem constants (hardcoded per contract).
B_FULL, T, F, H, D = 64, 512, 64, 64 * 4, 64  # H=256
G3 = 3 * H  # 768
NCORES = 8
B = B_FULL // NCORES  # 8 per core

FP = mybir.dt.float32
FR = mybir.dt.float32r  # same bits as fp32; PE streams 1 row/cycle (vs 4 for fp32)


CHUNK = 8  # timesteps per xp-prefetch / out-ring chunk

_CACHE = {}


def _split_waits(nc, template, cap=1):
    """walrus codegen rejects instructions with more than one sync wait
    (ISA structs have a single wait slot). Move excess waits onto cloned
    same-engine NOPs inserted right before the instruction (same per-engine
    queue order, identical semantics)."""
    import copy

    nid = 0
    for f in nc.m.functions:
        for bb in f.blocks:
            new = []
            for inst in bb.instructions:
                si = inst.sync_info
                waits = list(si.on_wait) if (si and si.on_wait) else []
                if len(waits) > cap and type(inst).__name__ != "InstNoOp":
                    for w in waits[:-cap]:
                        nid += 1
                        nop = copy.deepcopy(template)
                        nop.name = f"I-wsplit-{nid}"
                        nop.engine = inst.engine
                        nop.sync_info = mybir.SyncInfo(on_wait=[w], on_update=[])
                        new.append(nop)
                    inst.sync_info = mybir.SyncInfo(
                        on_wait=waits[-cap:],
                        on_update=list(si.on_update) if si.on_update else [],
                    )
                new.append(inst)
            bb.instructions = new


def _phase1(nc, tc, x_d, w_aug, i128, ones_fr, xp_dram):
    """xproj = x @ kernel + bias -> xp_dram[t, b, :]."""
    with (
        tc.tile_pool(name="p1_in", bufs=3) as p1_in,
        tc.tile_pool(name="p1_xt", bufs=3) as p1_xt,
        tc.tile_pool(name="p1_ps", bufs=2, space="PSUM") as p1_ps,
        tc.tile_pool(name="p1_pst", bufs=2, space="PSUM") as p1_pst,
        tc.tile_pool(name="p1_st", bufs=3) as p1_st,
    ):
        for b in range(B):
            for mt in range(T // 128):
                t0 = mt * 128
                xtile = p1_in.tile([128, F], FR)
                nc.sync.dma_start(xtile[:, :], x_d[b, t0 : t0 + 128, :])
                ps_t = p1_pst.tile([F, 128], FR)
                nc.tensor.transpose(ps_t[:, :], xtile[:, :], i128[:, :])
                xt_aug = p1_xt.tile([F + 1, 128], FR)
                nc.vector.tensor_copy(xt_aug[:F, :], ps_t[:, :])
                nc.vector.tensor_copy(xt_aug[F : F + 1, :], ones_fr[:, :])
                ps_xp = p1_ps.tile([128, G3], FP)
                nc.tensor.matmul(
                    ps_xp[:, 0:512], xt_aug[:, :], w_aug[:, 0:512],
                    start=True, stop=True,
                )
                nc.tensor.matmul(
                    ps_xp[:, 512:G3], xt_aug[:, :], w_aug[:, 512:G3],
                    start=True, stop=True,
                )
                xp_sb = p1_st.tile([128, G3], FR)
                nc.vector.tensor_copy(xp_sb[:, :], ps_xp[:, :])
                nc.sync.dma_start(xp_dram[t0 : t0 + 128, b, :], xp_sb[:, :])


def _phase2(nc, tc, xp_dram, r0, r1, i8f, i8r, hT, out_d, probe):
    """The 512-step recurrence."""
    no_mm = "no_mm" in probe
    no_gates = "no_gates" in probe
    no_tr = "no_tr" in probe
    with (
        tc.tile_pool(name="xp", bufs=2) as xp_pool,
        tc.tile_pool(name="ring", bufs=2) as ring_pool,
        tc.tile_pool(name="gates", bufs=2) as gates,
        tc.tile_pool(name="ps_rec", bufs=2, space="PSUM") as ps_rec_pool,
        tc.tile_pool(name="ps_sm", bufs=2, space="PSUM") as ps_sm_pool,
    ):
        prev_h = None
        for c in range(T // CHUNK):
            t0 = c * CHUNK
            xp_tile = xp_pool.tile([B, CHUNK, G3], FR)
            src = bass.AP(
                tensor=xp_dram.tensor,
                offset=xp_dram.offset + t0 * B * G3,
                ap=[[G3, B], [B * G3, CHUNK], [1, G3]],
            )
            nc.sync.dma_start(xp_tile[:, :, :], src)
            ring = ring_pool.tile([B, CHUNK, H], FP)

            for tl in range(CHUNK):
                t = t0 + tl
                first = t == 0
                ps = ps_rec_pool.tile([B, G3], FP)
                if not no_mm:
                    if not first:
                        # z,r banks first so sigmoid can start while the
                        # candidate-gate matmuls still stream
                        nc.tensor.matmul(
                            ps[:, 0:512], hT[:, 0:B], r0[:, 0:512],
                            start=True, stop=False,
                        )
                        nc.tensor.matmul(
                            ps[:, 0:512], hT[:, B : 2 * B], r1[:, 0:512],
                            start=False, stop=False,
                        )
                        nc.tensor.matmul(
                            ps[:, 0:512], i8r[:, :], xp_tile[:, tl, 0:512],
                            start=False, stop=True,
                        )
                        nc.tensor.matmul(
                            ps[:, 512:G3], hT[:, 0:B], r0[:, 512:G3],
                            start=True, stop=False,
                        )
                        nc.tensor.matmul(
                            ps[:, 512:G3], hT[:, B : 2 * B], r1[:, 512:G3],
                            start=False, stop=True,
                        )
                    else:
                        nc.tensor.matmul(
                            ps[:, 0:512], i8r[:, :], xp_tile[:, tl, 0:512],
                            start=True, stop=True,
                        )

                h_out = ring[:, tl, :]
                if not no_gates:
                    zr = gates.tile([B, 512], FP)
                    nc.scalar.activation(
                        zr[:, :], ps[:, 0:512],
                        mybir.ActivationFunctionType.Sigmoid,
                    )
                    tt = gates.tile([B, H], FP)
                    if not first:
                        v = gates.tile([B, H], FP)
                        nc.vector.tensor_mul(v[:, :], zr[:, H:512], ps[:, 512:G3])
                        w = gates.tile([B, H], FP)
                        nc.vector.tensor_add(
                            w[:, :], v[:, :], xp_tile[:, tl, 512:G3]
                        )
                        nc.scalar.activation(
                            tt[:, :], w[:, :], mybir.ActivationFunctionType.Tanh
                        )
                    else:
                        nc.scalar.activation(
                            tt[:, :], xp_tile[:, tl, 512:G3],
                            mybir.ActivationFunctionType.Tanh,
                        )
                    d = gates.tile([B, H], FP)
                    if first:
                        nc.vector.tensor_scalar_mul(d[:, :], tt[:, :], -1.0)
                    else:
                        nc.vector.tensor_sub(d[:, :], prev_h, tt[:, :])
                    e = gates.tile([B, H], FP)
                    nc.vector.tensor_mul(e[:, :], zr[:, 0:H], d[:, :])
                    nc.vector.tensor_add(h_out, e[:, :], tt[:, :])
                else:
                    nc.vector.memset(h_out, 0.0)
                prev_h = h_out

                if not no_tr:
                    ps_t = ps_sm_pool.tile([128, 2 * B], FP)
                    nc.tensor.transpose(ps_t[:, 0:B], ring[:, tl, 0:128], i8f[:, :])
                    nc.tensor.transpose(
                        ps_t[:, B : 2 * B], ring[:, tl, 128:256], i8f[:, :]
                    )
                    nc.scalar.copy(hT[:, :], ps_t[:, :])

            nc.sync.dma_start(out_d[:, t0 : t0 + CHUNK, :], ring[:, :, :])


def _phase3(nc, tc, statep, hT, i8r, dw0, dw1, db_bc, st_d, ps_pool):
    ps_st = ps_pool.tile([B, D], FP, tag="ps_state")
    nc.tensor.matmul(ps_st[:, :], hT[:, 0:B], dw0[:, :], start=True, stop=False)
    nc.tensor.matmul(
        ps_st[:, :], hT[:, B : 2 * B], dw1[:, :], start=False, stop=False
    )
    nc.tensor.matmul(ps_st[:, :], i8r[:, :], db_bc[:, :], start=False, stop=True)
    st_sb = statep.tile([B, D], FP)
    nc.scalar.activation(st_sb[:, :], ps_st[:, :], mybir.ActivationFunctionType.Tanh)
    nc.sync.dma_start(st_d[:, :], st_sb[:, :])


def _build_bass(probe=None):
    """probe: None for the real kernel, or a set of flags for TimelineSim
    bisection: {'no_p1', 'no_p2', 'no_mm', 'no_gates', 'no_tr'}."""
    probe = probe or set()
    nc = bass.Bass()

    x_d = nc.dram_tensor("x", [B, T, F], FR, kind="ExternalInput")
    k_d = nc.dram_tensor("kernel", [F, G3], FR, kind="ExternalInput")
    r_d = nc.dram_tensor("rkernel", [H, G3], FR, kind="ExternalInput")
    b_d = nc.dram_tensor("bias", [1, G3], FR, kind="ExternalInput")
    dw_d = nc.dram_tensor("dense_w", [H, D], FR, kind="ExternalInput")
    db_d = nc.dram_tensor("dense_b", [1, D], FR, kind="ExternalInput")

    out_d = nc.dram_tensor("out", [B, T, H], FP, kind="ExternalOutput")
    st_d = nc.dram_tensor("state", [B, D], FP, kind="ExternalOutput")

    with ExitStack() as ctx:
        tc = ctx.enter_context(tile.TileContext(nc))
        nop_template = nc.tensor.nop().ins
        consts = ctx.enter_context(tc.tile_pool(name="consts", bufs=1))
        statep = ctx.enter_context(tc.tile_pool(name="statep", bufs=1))
        dram = ctx.enter_context(tc.tile_pool(name="dram", bufs=1, space="DRAM"))

        i128f = consts.tile([128, 128], FP)
        make_identity(nc, i128f)
        i128 = consts.tile([128, 128], FR)
        nc.vector.tensor_copy(i128[:, :], i128f[:, :])
        i8f = consts.tile([8, 8], FP)
        make_identity(nc, i8f)
        i8r = consts.tile([8, 8], FR)
        nc.vector.tensor_copy(i8r[:, :], i8f[:, :])

        w_aug = consts.tile([F + 1, G3], FR)
        nc.sync.dma_start(w_aug[:F, :], k_d[:, :])
        nc.sync.dma_start(w_aug[F : F + 1, :], b_d[:, :])

        r0 = consts.tile([128, G3], FR)
        r1 = consts.tile([128, G3], FR)
        nc.sync.dma_start(r0[:, :], r_d[0:128, :])
        nc.sync.dma_start(r1[:, :], r_d[128:256, :])

        dw0 = consts.tile([128, D], FR)
        dw1 = consts.tile([128, D], FR)
        nc.sync.dma_start(dw0[:, :], dw_d[0:128, :])
        nc.sync.dma_start(dw1[:, :], dw_d[128:256, :])
        db_bc = consts.tile([B, D], FR)
        nc.sync.dma_start(
            db_bc[:, :], bass.AP(tensor=db_d, offset=0, ap=[[0, B], [1, D]])
        )

        zf = consts.tile([128, 2 * B], FP)
        nc.vector.memset(zf[:, :], 0.0)
        hT = statep.tile([128, 2 * B], FR)
        nc.vector.tensor_copy(hT[:, :], zf[:, :])
        onesf = consts.tile([1, 128], FP)
        nc.vector.memset(onesf[:, :], 1.0)
        ones_fr = consts.tile([1, 128], FR)
        nc.vector.tensor_copy(ones_fr[:, :], onesf[:, :])

        xp_dram = dram.tile([T, B, G3], FR)

        if "no_p1" not in probe:
            _phase1(nc, tc, x_d, w_aug, i128, ones_fr, xp_dram)

        if "no_p2" not in probe:
            _phase2(nc, tc, xp_dram, r0, r1, i8f, i8r, hT, out_d, probe)

        with tc.tile_pool(name="ps_p3", bufs=1, space="PSUM") as ps_p3:
            _phase3(nc, tc, statep, hT, i8r, dw0, dw1, db_bc, st_d, ps_p3)

    _split_waits(nc, nop_template)
    return nc


def _get_runner():
    """Build nc + a cached jitted shard_map executable (no output donation so
    the zero-output buffers can be reused across timing reps)."""
    if "runner" in _CACHE:
        return _CACHE["runner"]
    import jax
    from jax.experimental.shard_map import shard_map
    from jax.sharding import Mesh, PartitionSpec

    from concourse import bass2jax as b2j

    nc = _CACHE.setdefault("nc", _build_bass())
    b2j.install_neuronx_cc_hook()

    in_names, out_names, out_avals, zero_outs = [], [], [], []
    partition_name = nc.partition_id_tensor.name if nc.partition_id_tensor else None
    for alloc in nc.m.functions[0].allocations:
        if not isinstance(alloc, mybir.MemoryLocationSet):
            continue
        name = alloc.memorylocations[0].name
        if alloc.kind == "ExternalInput":
            if name != partition_name:
                in_names.append(name)
        elif alloc.kind == "ExternalOutput":
            shape = tuple(alloc.tensor_shape)
            dtype = mybir.dt.np(alloc.dtype)
            out_names.append(name)
            out_avals.append(jax.core.ShapedArray(shape, dtype))
            zero_outs.append(np.zeros(shape, dtype))
    n_params = len(in_names)
    all_in_names = in_names + out_names
    if partition_name is not None:
        all_in_names.append(partition_name)

    def _body(*args):
        operands = list(args)
        if partition_name is not None:
            operands.append(b2j.partition_id_tensor())
        outs = b2j._bass_exec_p.bind(
            *operands,
            out_avals=tuple(out_avals),
            in_names=tuple(all_in_names),
            out_names=tuple(out_names),
            lowering_input_output_aliases=(),
            sim_require_finite=True,
            sim_require_nnan=True,
            nc=nc,
        )
        return tuple(outs)

    devices = jax.devices()[:NCORES]
    mesh = Mesh(np.asarray(devices), ("core",))
    n_outs = len(out_names)
    sharded = jax.jit(
        shard_map(
            _body,
            mesh=mesh,
            in_specs=(PartitionSpec("core"),) * (n_params + n_outs),
            out_specs=(PartitionSpec("core"),) * n_outs,
            check_rep=False,
        ),
        keep_unused=True,
    )
    runner = (sharded, in_names, out_names, zero_outs)
    _CACHE["runner"] = runner
    return runner


def _prep_inputs(x, kernel, recurrent_kernel, bias, dense_w, dense_b):
    x = np.ascontiguousarray(np.asarray(x, dtype=np.float32))
    k = np.ascontiguousarray(np.asarray(kernel, dtype=np.float32))
    rk = np.ascontiguousarray(np.asarray(recurrent_kernel, dtype=np.float32))
    bi = np.ascontiguousarray(np.asarray(bias, dtype=np.float32).reshape(1, G3))
    dw = np.ascontiguousarray(np.asarray(dense_w, dtype=np.float32))
    db = np.ascontiguousarray(np.asarray(dense_b, dtype=np.float32).reshape(1, D))
    return {
        "x": x.reshape(B_FULL, T, F),  # sharded over axis0 by shard_map
        "kernel": np.concatenate([k] * NCORES, axis=0),
        "rkernel": np.concatenate([rk] * NCORES, axis=0),
        "bias": np.concatenate([bi] * NCORES, axis=0),
        "dense_w": np.concatenate([dw] * NCORES, axis=0),
        "dense_b": np.concatenate([db] * NCORES, axis=0),
    }


def _run(concat_inputs):
    sharded, in_names, out_names, zero_outs = _get_runner()
    zeros_cat = [
        np.zeros((NCORES * z.shape[0], *z.shape[1:]), z.dtype) for z in zero_outs
    ]
    args = [concat_inputs[n] for n in in_names] + zeros_cat
    outs = sharded(*args)
    return dict(zip(out_names, (np.asarray(o) for o in outs)))


def kernel(x, kernel, recurrent_kernel, bias, dense_w, dense_b):
    concat = _prep_inputs(x, kernel, recurrent_kernel, bias, dense_w, dense_b)
    res = _run(concat)
    return res["out"], res["state"]


def timed_run(inputs, reps=10):
    """Time device-resident repeated executions; returns (best_ns, list_ns)."""
    import time as _time

    import jax

    concat = _prep_inputs(**inputs)
    sharded, in_names, out_names, zero_outs = _get_runner()
    zeros_cat = [
        np.zeros((NCORES * z.shape[0], *z.shape[1:]), z.dtype) for z in zero_outs
    ]
    args = [concat[n] for n in in_names] + zeros_cat
    dev_args = jax.device_put(args)
    jax.block_until_ready(dev_args)
    jax.block_until_ready(sharded(*dev_args))
    times = []
    for _ in range(reps):
        t0 = _time.perf_counter()
        jax.block_until_ready(sharded(*dev_args))
        times.append((_time.perf_counter() - t0) * 1e9)
    return min(times), times
